# revision 9
# baseline (speedup 1.0000x reference)
"""Expert-parallel MoE kernel for Trainium2 (8 NeuronCores, one expert per core).

T=2048 tokens, H=2048 hidden, I=1408 intermediate, E=8 experts, top-2 routing
with DeepSeek group mask (4 groups of 2 experts, top-2 groups), fake-quant
(per-1x128-tile int8 absmax) on dispatch tokens and combine outputs.

Per-core SPMD inputs:
  tokens [T,H], tokensT [H,T], wr [H,E]  (replicated)
  wg/wu [H,I], wd [I,H]                  (this core's expert slices)
  esel1 [1,E]                            (one-hot of this core's expert)
Outputs: outp [T,H] partial (weighted quantized expert output), aux [1,1].
Host: sum the 8 partials; take core 0's aux.

Numerics: router GEMM in exact fp32; expert GEMMs in f32r (11-bit mantissa,
full PE rate); all routing/selection decisions on fp32 logits (sigmoid is
monotone); quant round via +/-1.5*2^23 magic (round-half-even, matches
jnp.round); no int8 anywhere (HW cast truncates).
"""
import numpy as np

import concourse.bass as bass
import concourse.tile as tile
import concourse.mybir as mybir
from concourse import bacc
from concourse.bass_utils import run_bass_kernel_spmd
from concourse.masks import make_identity

f32 = mybir.dt.float32
f32r = mybir.dt.float32r
AF = mybir.ActivationFunctionType
ALU = mybir.AluOpType
AX = mybir.AxisListType

T, H, I, E = 2048, 2048, 1408, 8
NG, TOPK = 4, 2
QMAX = 127.0
MAGIC = 12582912.0          # 1.5 * 2**23
NT, NH, NI = T // 128, H // 128, I // 128
TCH = 1024                  # token chunk for stages 1-3
NCH = T // TCH
NSUB = TCH // 512


def bcast(ap, n, where):
    """Read-broadcast a [P, a] AP along a new free dim of size n."""
    p, a = ap.ap[0], ap.ap[1]
    new = [p, [0, n], a] if where == "outer" else [p, a, [0, n]]
    return bass.AP(tensor=ap.tensor, offset=ap.offset, ap=new)


def build():
    nc = bacc.Bacc("TRN2", target_bir_lowering=False, debug=False, num_devices=8)

    tokens = nc.dram_tensor("tokens", [T, H], f32, kind="ExternalInput")
    tokensT = nc.dram_tensor("tokensT", [H, T], f32, kind="ExternalInput")
    wr = nc.dram_tensor("wr", [H, E], f32, kind="ExternalInput")
    wg = nc.dram_tensor("wg", [H, I], f32, kind="ExternalInput")
    wu = nc.dram_tensor("wu", [H, I], f32, kind="ExternalInput")
    wd = nc.dram_tensor("wd", [I, H], f32, kind="ExternalInput")
    esel1 = nc.dram_tensor("esel1", [1, E], f32, kind="ExternalInput")
    outp = nc.dram_tensor("outp", [T, H], f32, kind="ExternalOutput")
    aux = nc.dram_tensor("aux", [1, 1], f32, kind="ExternalOutput")

    with tile.TileContext(nc) as tc:
        with tc.tile_pool(name="const", bufs=1) as constp, \
             tc.tile_pool(name="persist", bufs=1) as persist:

            ident = constp.tile([128, 128], f32)
            make_identity(nc, ident)
            ones = constp.tile([128, 1], f32)
            nc.vector.memset(ones, 1.0)
            w_all = persist.tile([128, NT], f32)   # routing weight per token

            # ---------------- Phase 0: router + selection + aux ----------------
            with tc.tile_pool(name="p0", bufs=2) as p0, \
                 tc.tile_pool(name="p0sel", bufs=3) as p0sel, \
                 tc.tile_pool(name="p0ps", bufs=1, space="PSUM") as p0ps:
                neg = p0.tile([128, E], f32, tag="negt", bufs=1)
                nc.vector.memset(neg, -1e30)
                eselb = p0.tile([128, E], f32, tag="eselb", bufs=1)
                nc.sync.dma_start(
                    out=eselb,
                    in_=bass.AP(tensor=esel1, offset=0, ap=[[0, 128], [1, E]]))
                wrt = p0.tile([128, NH, E], f32, tag="wrt", bufs=1)
                nc.sync.dma_start(
                    out=wrt, in_=wr[:, :].rearrange("(a p) e -> p a e", p=128))

                aux_ps = p0ps.tile([1, 16], f32, tag="auxps")
                for sub in range(T // 512):
                    ps_log = p0ps.tile([8, 512], f32, tag="pslog")
                    for h in range(NH):
                        strip = p0.tile([128, 512], f32, tag="strip")
                        nc.sync.dma_start(
                            out=strip,
                            in_=tokensT[h * 128:(h + 1) * 128,
                                        sub * 512:(sub + 1) * 512])
                        nc.tensor.matmul(ps_log, wrt[:, h, :], strip,
                                         start=(h == 0), stop=(h == NH - 1))
                    logT = p0.tile([8, 512], f32, tag="logT")
                    nc.vector.tensor_copy(logT, ps_log)
                    for k in range(4):
                        tt = sub * 4 + k
                        ptl = p0ps.tile([128, 8], f32, tag="ptl")
                        nc.tensor.transpose(ptl, logT[:, k * 128:(k + 1) * 128],
                                            ident[0:8, 0:8])
                        lg = p0sel.tile([128, E], f32, tag="lg")
                        nc.vector.tensor_copy(lg, ptl)

                        glog = p0sel.tile([128, NG], f32, tag="glog")
                        nc.vector.tensor_reduce(
                            glog, lg.rearrange("p (g b) -> p g b", b=2),
                            axis=AX.X, op=ALU.max)
                        tg = p0sel.tile([128, NG, NG], f32, tag="tg")
                        nc.vector.tensor_tensor(tg, bcast(glog, NG, "outer"),
                                                bcast(glog, NG, "inner"),
                                                op=ALU.is_gt)
                        gcnt = p0sel.tile([128, NG], f32, tag="gcnt")
                        nc.vector.tensor_reduce(gcnt, tg, axis=AX.X, op=ALU.add)
                        gsel = p0sel.tile([128, NG], mybir.dt.uint8, tag="gsel")
                        nc.vector.tensor_single_scalar(gsel, gcnt, 1.5,
                                                       op=ALU.is_lt)
                        mlog = p0sel.tile([128, E], f32, tag="mlog")
                        nc.vector.select(
                            mlog.rearrange("p (g b) -> p g b", b=2),
                            bcast(gsel, 2, "inner"),
                            lg.rearrange("p (g b) -> p g b", b=2),
                            neg.rearrange("p (g b) -> p g b", b=2))
                        te = p0sel.tile([128, E, E], f32, tag="te")
                        nc.vector.tensor_tensor(te, bcast(mlog, E, "outer"),
                                                bcast(mlog, E, "inner"),
                                                op=ALU.is_gt)
                        ecnt = p0sel.tile([128, E], f32, tag="ecnt")
                        nc.vector.tensor_reduce(ecnt, te, axis=AX.X, op=ALU.add)
                        esel = p0sel.tile([128, E], f32, tag="esel")
                        nc.vector.tensor_single_scalar(esel, ecnt, 1.5,
                                                       op=ALU.is_lt)

                        scores = p0sel.tile([128, E], f32, tag="scores")
                        nc.scalar.activation(scores, lg, AF.Sigmoid)
                        wun = p0sel.tile([128, E], f32, tag="wun")
                        nc.vector.tensor_mul(wun, scores, esel)
                        denom = p0sel.tile([128, 1], f32, tag="denom")
                        nc.vector.tensor_reduce(denom, wun, axis=AX.X, op=ALU.add)
                        nc.vector.tensor_scalar_max(denom, denom, 1e-9)
                        rden = p0sel.tile([128, 1], f32, tag="rden")
                        nc.vector.reciprocal(rden, denom)
                        wfull = p0sel.tile([128, E], f32, tag="wfull")
                        nc.vector.tensor_scalar(wfull, wun, rden, None,
                                                op0=ALU.mult)
                        wsel = p0sel.tile([128, E], f32, tag="wsel")
                        nc.vector.tensor_mul(wsel, wfull, eselb)
                        nc.vector.tensor_reduce(w_all[:, tt:tt + 1], wsel,
                                                axis=AX.X, op=ALU.add)

                        rsum = p0sel.tile([128, 1], f32, tag="rsum")
                        nc.vector.tensor_reduce(rsum, scores, axis=AX.X,
                                                op=ALU.add)
                        nc.vector.tensor_scalar_max(rsum, rsum, 1e-9)
                        rr = p0sel.tile([128, 1], f32, tag="rr")
                        nc.vector.reciprocal(rr, rsum)
                        cat16 = p0sel.tile([128, 16], f32, tag="cat16")
                        nc.vector.tensor_scalar(cat16[:, 0:8], scores, rr, None,
                                                op0=ALU.mult)
                        nc.vector.tensor_copy(cat16[:, 8:16], esel)
                        nc.tensor.matmul(aux_ps, ones, cat16,
                                         start=(tt == 0), stop=(tt == NT - 1))

                auxsb = p0sel.tile([1, 16], f32, tag="auxsb")
                nc.vector.tensor_copy(auxsb, aux_ps[0:1, :])
                auxtmp = p0sel.tile([1, E], f32, tag="auxtmp")
                nc.vector.tensor_mul(auxtmp, auxsb[0:1, 0:8], auxsb[0:1, 8:16])
                auxs = p0sel.tile([1, 1], f32, tag="auxs")
                nc.vector.tensor_reduce(auxs, auxtmp, axis=AX.X, op=ALU.add)
                nc.vector.tensor_scalar_mul(auxs, auxs,
                                            float(E) / (T * T * TOPK))
                nc.sync.dma_start(out=aux[:, :], in_=auxs)

            # ---------------- Phases A/B/C per token chunk ----------------
            with tc.tile_pool(name="big", bufs=1) as gpool, \
                 tc.tile_pool(name="io", bufs=2) as bigio, \
                 tc.tile_pool(name="wp", bufs=2) as wpool, \
                 tc.tile_pool(name="sm", bufs=4) as smalls, \
                 tc.tile_pool(name="post", bufs=3) as post, \
                 tc.tile_pool(name="pstr", bufs=2, space="PSUM") as ps_tr, \
                 tc.tile_pool(name="psmm", bufs=2, space="PSUM") as ps_mm, \
                 tc.tile_pool(name="psz", bufs=2, space="PSUM") as ps_zp:
                for c in range(NCH):
                    t0 = c * TCH
                    # --- A: quantize (exact fp32, in place) + PE-transpose ---
                    xqT = gpool.tile([128, NH, TCH], f32r, tag="xqT")
                    for ttl in range(TCH // 128):
                        tt = t0 // 128 + ttl
                        tok = bigio.tile([128, H], f32, tag="tok")
                        nc.sync.dma_start(
                            out=tok, in_=tokens[tt * 128:(tt + 1) * 128, :])
                        amax = smalls.tile([128, NH], f32, tag="amax")
                        nc.vector.tensor_reduce(
                            amax, tok.rearrange("p (a b) -> p a b", b=128),
                            axis=AX.X, op=ALU.max, apply_absolute_value=True)
                        scl = smalls.tile([128, NH], f32, tag="scl")
                        nc.vector.tensor_scalar_mul(scl, amax, float(1.0 / QMAX))
                        rs = smalls.tile([128, NH], f32, tag="rs")
                        nc.vector.reciprocal(rs, scl)
                        for j in range(NH):
                            nc.scalar.activation(tok[:, j * 128:(j + 1) * 128],
                                                 tok[:, j * 128:(j + 1) * 128],
                                                 AF.Copy, bias=MAGIC,
                                                 scale=rs[:, j:j + 1])
                        nc.vector.tensor_scalar_add(tok, tok, -MAGIC)
                        for j in range(NH):
                            nc.scalar.activation(tok[:, j * 128:(j + 1) * 128],
                                                 tok[:, j * 128:(j + 1) * 128],
                                                 AF.Copy, scale=scl[:, j:j + 1])
                        for j in range(NH):
                            ptr = ps_tr.tile([128, 128], f32, tag="ptr")
                            nc.tensor.transpose(
                                ptr, tok[:, j * 128:(j + 1) * 128], ident)
                            nc.vector.tensor_copy(
                                xqT[:, j, ttl * 128:(ttl + 1) * 128], ptr)

                    # --- B: gate/up GEMMs (f32r) + SwiGLU -> gatedT ---
                    gatedT = gpool.tile([128, NI, TCH], f32r, tag="gatedT")
                    for i in range(NI):
                        wgi = wpool.tile([128, NH, 128], f32, tag="wgi")
                        nc.sync.dma_start(
                            out=wgi,
                            in_=wg[:, i * 128:(i + 1) * 128].rearrange(
                                "(a p) c -> p a c", p=128))
                        wgr = wpool.tile([128, NH, 128], f32r, tag="wgr")
                        nc.vector.tensor_copy(wgr, wgi)
                        pg = []
                        for sub in range(NSUB):
                            s0 = sub * 512
                            ps_g = ps_mm.tile([128, 512], f32, tag="ps_g")
                            for h in range(NH):
                                nc.tensor.matmul(ps_g, wgr[:, h, :],
                                                 xqT[:, h, s0:s0 + 512],
                                                 start=(h == 0),
                                                 stop=(h == NH - 1))
                            pg.append(ps_g)
                        wui = wpool.tile([128, NH, 128], f32, tag="wgi")
                        nc.sync.dma_start(
                            out=wui,
                            in_=wu[:, i * 128:(i + 1) * 128].rearrange(
                                "(a p) c -> p a c", p=128))
                        wur = wpool.tile([128, NH, 128], f32r, tag="wgr")
                        nc.scalar.copy(wur, wui)
                        for sub in range(NSUB):
                            s0 = sub * 512
                            ps_u = ps_mm.tile([128, 512], f32, tag="ps_u")
                            for h in range(NH):
                                nc.tensor.matmul(ps_u, wur[:, h, :],
                                                 xqT[:, h, s0:s0 + 512],
                                                 start=(h == 0),
                                                 stop=(h == NH - 1))
                            sil = post.tile([128, 512], f32, tag="sil")
                            nc.scalar.activation(sil, pg[sub], AF.Silu)
                            nc.vector.tensor_mul(gatedT[:, i, s0:s0 + 512],
                                                 sil, ps_u)

                    # --- C: down GEMM + combine quant + routing weight ---
                    for hc in range(H // 512):
                        wdq = []
                        for i in range(NI):
                            wdi = wpool.tile([128, 512], f32, tag="wdi",
                                             bufs=2)
                            nc.sync.dma_start(
                                out=wdi,
                                in_=wd[i * 128:(i + 1) * 128,
                                       hc * 512:(hc + 1) * 512])
                            wdr = wpool.tile([128, 512], f32r, tag="wdr",
                                             bufs=NI + 1)
                            if i % 2 == 0:
                                nc.vector.tensor_copy(wdr, wdi)
                            else:
                                nc.scalar.copy(wdr, wdi)
                            wdq.append(wdr)
                        for ttl in range(TCH // 128):
                            tt = t0 // 128 + ttl
                            psz = ps_zp.tile([128, 512], f32, tag="pszt")
                            for i in range(NI):
                                nc.tensor.matmul(
                                    psz,
                                    gatedT[:, i, ttl * 128:(ttl + 1) * 128],
                                    wdq[i],
                                    start=(i == 0), stop=(i == NI - 1))
                            zamax = smalls.tile([128, 4], f32, tag="zamax")
                            nc.vector.tensor_reduce(
                                zamax, psz.rearrange("p (a b) -> p a b", b=128),
                                axis=AX.X, op=ALU.max, apply_absolute_value=True)
                            zscl = smalls.tile([128, 4], f32, tag="zscl")
                            nc.vector.tensor_scalar_mul(zscl, zamax,
                                                        float(1.0 / QMAX))
                            zrs = smalls.tile([128, 4], f32, tag="zrs")
                            nc.vector.reciprocal(zrs, zscl)
                            zsw = smalls.tile([128, 4], f32, tag="zsw")
                            nc.vector.tensor_scalar(zsw, zscl,
                                                    w_all[:, tt:tt + 1],
                                                    None, op0=ALU.mult)
                            rz = post.tile([128, 512], f32, tag="rz")
                            for j in range(4):
                                nc.scalar.activation(
                                    rz[:, j * 128:(j + 1) * 128],
                                    psz[:, j * 128:(j + 1) * 128],
                                    AF.Copy, bias=MAGIC, scale=zrs[:, j:j + 1])
                            nc.vector.tensor_scalar_add(rz, rz, -MAGIC)
                            for j in range(4):
                                nc.scalar.activation(
                                    rz[:, j * 128:(j + 1) * 128],
                                    rz[:, j * 128:(j + 1) * 128],
                                    AF.Copy, scale=zsw[:, j:j + 1])
                            nc.sync.dma_start(
                                out=outp[tt * 128:(tt + 1) * 128,
                                         hc * 512:(hc + 1) * 512],
                                in_=rz)

    nc.compile()
    return nc


_NC_CACHE = {}
_LAST = {"exec_ns": None}
TRACE = False


def last_exec_ns():
    return _LAST["exec_ns"]


def kernel(tokens, w_router, w_gate, w_up, w_down):
    tokens = np.ascontiguousarray(tokens, dtype=np.float32)
    w_router = np.ascontiguousarray(w_router, dtype=np.float32)
    tokensT = np.ascontiguousarray(tokens.T)

    if "nc" not in _NC_CACHE:
        _NC_CACHE["nc"] = build()
    nc = _NC_CACHE["nc"]

    in_maps = []
    for e in range(E):
        onehot = np.zeros((1, E), dtype=np.float32)
        onehot[0, e] = 1.0
        in_maps.append(dict(
            tokens=tokens, tokensT=tokensT, wr=w_router,
            wg=np.ascontiguousarray(w_gate[e], dtype=np.float32),
            wu=np.ascontiguousarray(w_up[e], dtype=np.float32),
            wd=np.ascontiguousarray(w_down[e], dtype=np.float32),
            esel1=onehot))

    res = run_bass_kernel_spmd(nc, in_maps, core_ids=list(range(E)),
                               trace=TRACE)
    if res.exec_time_ns is not None:
        _LAST["exec_ns"] = res.exec_time_ns
    out = np.zeros((T, H), dtype=np.float64)
    for e in range(E):
        out += res.results[e]["outp"].astype(np.float64)
    aux = np.float32(res.results[0]["aux"][0, 0])
    return out.astype(np.float32), aux


# revision 11
# speedup vs baseline: 21.7665x; 21.7665x over previous
"""Expert-parallel MoE kernel for Trainium2 (8 NeuronCores, one expert per core).

T=2048 tokens, H=2048 hidden, I=1408 intermediate, E=8 experts, top-2 routing
with DeepSeek group mask (4 groups of 2 experts, top-2 groups), fake-quant
(per-1x128-tile int8 absmax) on dispatch tokens and combine outputs.

Per-core SPMD inputs:
  tokens [T,H], tokensT [H,T], wr [H,E]  (replicated)
  wg/wu [H,I], wd [I,H]                  (this core's expert slices)
  esel1 [1,E]                            (one-hot of this core's expert)
Outputs: outp [T,H] partial (weighted quantized expert output), aux [1,1].
Host: sum the 8 partials; take core 0's aux.

Numerics: router GEMM in exact fp32; expert GEMMs in f32r (11-bit mantissa,
full PE rate); all routing/selection decisions on fp32 logits (sigmoid is
monotone); quant round via +/-1.5*2^23 magic (round-half-even, matches
jnp.round); no int8 anywhere (HW cast truncates).
"""
import numpy as np

import concourse.bass as bass
import concourse.tile as tile
import concourse.mybir as mybir
from concourse import bacc
from concourse.bass_utils import run_bass_kernel_spmd
from concourse.masks import make_identity

f32 = mybir.dt.float32
f32r = mybir.dt.float32r
AF = mybir.ActivationFunctionType
ALU = mybir.AluOpType
AX = mybir.AxisListType

T, H, I, E = 2048, 2048, 1408, 8
NG, TOPK = 4, 2
QMAX = 127.0
MAGIC = 12582912.0          # 1.5 * 2**23
NT, NH, NI = T // 128, H // 128, I // 128
TCH = 1024                  # token chunk for stages 1-3
NCH = T // TCH
NSUB = TCH // 512


def bcast(ap, n, where):
    """Read-broadcast a [P, a] AP along a new free dim of size n."""
    p, a = ap.ap[0], ap.ap[1]
    new = [p, [0, n], a] if where == "outer" else [p, a, [0, n]]
    return bass.AP(tensor=ap.tensor, offset=ap.offset, ap=new)


def build(variant="full"):
    nc = bacc.Bacc("TRN2", target_bir_lowering=False, debug=False, num_devices=8)

    tokens = nc.dram_tensor("tokens", [T, H], f32, kind="ExternalInput")
    tokensT = nc.dram_tensor("tokensT", [H, T], f32, kind="ExternalInput")
    wr = nc.dram_tensor("wr", [H, E], f32, kind="ExternalInput")
    wg = nc.dram_tensor("wg", [H, I], f32, kind="ExternalInput")
    wu = nc.dram_tensor("wu", [H, I], f32, kind="ExternalInput")
    wd = nc.dram_tensor("wd", [I, H], f32, kind="ExternalInput")
    esel1 = nc.dram_tensor("esel1", [1, E], f32, kind="ExternalInput")
    outp = nc.dram_tensor("outp", [T, H], f32, kind="ExternalOutput")
    aux = nc.dram_tensor("aux", [1, 1], f32, kind="ExternalOutput")

    if variant == "noop":
        with tile.TileContext(nc) as tc:
            with tc.tile_pool(name="nop", bufs=1) as pool:
                t = pool.tile([128, 512], f32)
                nc.sync.dma_start(out=t, in_=tokens[0:128, 0:512])
                nc.vector.tensor_scalar_mul(t, t, 1.0)
                nc.sync.dma_start(out=outp[0:128, 0:512], in_=t)
                t2 = pool.tile([1, 1], f32)
                nc.vector.memset(t2, 0.0)
                nc.sync.dma_start(out=aux[:, :], in_=t2)
        nc.compile()
        return nc

    with tile.TileContext(nc) as tc:
        with tc.tile_pool(name="const", bufs=1) as constp, \
             tc.tile_pool(name="persist", bufs=1) as persist:

            ident = constp.tile([128, 128], f32)
            make_identity(nc, ident)
            ones = constp.tile([128, 1], f32)
            nc.vector.memset(ones, 1.0)
            w_all = persist.tile([128, NT], f32)   # routing weight per token

            # ---------------- Phase 0: router + selection + aux ----------------
            with tc.tile_pool(name="p0", bufs=2) as p0, \
                 tc.tile_pool(name="p0sel", bufs=3) as p0sel, \
                 tc.tile_pool(name="p0ps", bufs=1, space="PSUM") as p0ps:
                neg = p0.tile([128, E], f32, tag="negt", bufs=1)
                nc.vector.memset(neg, -1e30)
                eselb = p0.tile([128, E], f32, tag="eselb", bufs=1)
                nc.sync.dma_start(
                    out=eselb,
                    in_=bass.AP(tensor=esel1, offset=0, ap=[[0, 128], [1, E]]))
                wrt = p0.tile([128, NH, E], f32, tag="wrt", bufs=1)
                nc.sync.dma_start(
                    out=wrt, in_=wr[:, :].rearrange("(a p) e -> p a e", p=128))

                aux_ps = p0ps.tile([1, 16], f32, tag="auxps")
                for sub in range(T // 512):
                    ps_log = p0ps.tile([8, 512], f32, tag="pslog")
                    for h in range(NH):
                        strip = p0.tile([128, 512], f32, tag="strip")
                        nc.sync.dma_start(
                            out=strip,
                            in_=tokensT[h * 128:(h + 1) * 128,
                                        sub * 512:(sub + 1) * 512])
                        nc.tensor.matmul(ps_log, wrt[:, h, :], strip,
                                         start=(h == 0), stop=(h == NH - 1))
                    logT = p0.tile([8, 512], f32, tag="logT")
                    nc.vector.tensor_copy(logT, ps_log)
                    for k in range(4):
                        tt = sub * 4 + k
                        ptl = p0ps.tile([128, 8], f32, tag="ptl")
                        nc.tensor.transpose(ptl, logT[:, k * 128:(k + 1) * 128],
                                            ident[0:8, 0:8])
                        lg = p0sel.tile([128, E], f32, tag="lg")
                        nc.vector.tensor_copy(lg, ptl)

                        glog = p0sel.tile([128, NG], f32, tag="glog")
                        nc.vector.tensor_reduce(
                            glog, lg.rearrange("p (g b) -> p g b", b=2),
                            axis=AX.X, op=ALU.max)
                        tg = p0sel.tile([128, NG, NG], f32, tag="tg")
                        nc.vector.tensor_tensor(tg, bcast(glog, NG, "outer"),
                                                bcast(glog, NG, "inner"),
                                                op=ALU.is_gt)
                        gcnt = p0sel.tile([128, NG], f32, tag="gcnt")
                        nc.vector.tensor_reduce(gcnt, tg, axis=AX.X, op=ALU.add)
                        gsel = p0sel.tile([128, NG], mybir.dt.uint8, tag="gsel")
                        nc.vector.tensor_single_scalar(gsel, gcnt, 1.5,
                                                       op=ALU.is_lt)
                        mlog = p0sel.tile([128, E], f32, tag="mlog")
                        nc.vector.select(
                            mlog.rearrange("p (g b) -> p g b", b=2),
                            bcast(gsel, 2, "inner"),
                            lg.rearrange("p (g b) -> p g b", b=2),
                            neg.rearrange("p (g b) -> p g b", b=2))
                        te = p0sel.tile([128, E, E], f32, tag="te")
                        nc.vector.tensor_tensor(te, bcast(mlog, E, "outer"),
                                                bcast(mlog, E, "inner"),
                                                op=ALU.is_gt)
                        ecnt = p0sel.tile([128, E], f32, tag="ecnt")
                        nc.vector.tensor_reduce(ecnt, te, axis=AX.X, op=ALU.add)
                        esel = p0sel.tile([128, E], f32, tag="esel")
                        nc.vector.tensor_single_scalar(esel, ecnt, 1.5,
                                                       op=ALU.is_lt)

                        scores = p0sel.tile([128, E], f32, tag="scores")
                        nc.scalar.activation(scores, lg, AF.Sigmoid)
                        wun = p0sel.tile([128, E], f32, tag="wun")
                        nc.vector.tensor_mul(wun, scores, esel)
                        denom = p0sel.tile([128, 1], f32, tag="denom")
                        nc.vector.tensor_reduce(denom, wun, axis=AX.X, op=ALU.add)
                        nc.vector.tensor_scalar_max(denom, denom, 1e-9)
                        rden = p0sel.tile([128, 1], f32, tag="rden")
                        nc.vector.reciprocal(rden, denom)
                        wfull = p0sel.tile([128, E], f32, tag="wfull")
                        nc.vector.tensor_scalar(wfull, wun, rden, None,
                                                op0=ALU.mult)
                        wsel = p0sel.tile([128, E], f32, tag="wsel")
                        nc.vector.tensor_mul(wsel, wfull, eselb)
                        nc.vector.tensor_reduce(w_all[:, tt:tt + 1], wsel,
                                                axis=AX.X, op=ALU.add)

                        rsum = p0sel.tile([128, 1], f32, tag="rsum")
                        nc.vector.tensor_reduce(rsum, scores, axis=AX.X,
                                                op=ALU.add)
                        nc.vector.tensor_scalar_max(rsum, rsum, 1e-9)
                        rr = p0sel.tile([128, 1], f32, tag="rr")
                        nc.vector.reciprocal(rr, rsum)
                        cat16 = p0sel.tile([128, 16], f32, tag="cat16")
                        nc.vector.tensor_scalar(cat16[:, 0:8], scores, rr, None,
                                                op0=ALU.mult)
                        nc.vector.tensor_copy(cat16[:, 8:16], esel)
                        nc.tensor.matmul(aux_ps, ones, cat16,
                                         start=(tt == 0), stop=(tt == NT - 1))

                auxsb = p0sel.tile([1, 16], f32, tag="auxsb")
                nc.vector.tensor_copy(auxsb, aux_ps[0:1, :])
                auxtmp = p0sel.tile([1, E], f32, tag="auxtmp")
                nc.vector.tensor_mul(auxtmp, auxsb[0:1, 0:8], auxsb[0:1, 8:16])
                auxs = p0sel.tile([1, 1], f32, tag="auxs")
                nc.vector.tensor_reduce(auxs, auxtmp, axis=AX.X, op=ALU.add)
                nc.vector.tensor_scalar_mul(auxs, auxs,
                                            float(E) / (T * T * TOPK))
                nc.sync.dma_start(out=aux[:, :], in_=auxs)

            # ---------------- Phases A/B/C per token chunk ----------------
            with tc.tile_pool(name="big", bufs=1) as gpool, \
                 tc.tile_pool(name="io", bufs=2) as bigio, \
                 tc.tile_pool(name="wp", bufs=2) as wpool, \
                 tc.tile_pool(name="sm", bufs=4) as smalls, \
                 tc.tile_pool(name="post", bufs=3) as post, \
                 tc.tile_pool(name="pstr", bufs=2, space="PSUM") as ps_tr, \
                 tc.tile_pool(name="psmm", bufs=2, space="PSUM") as ps_mm, \
                 tc.tile_pool(name="psz", bufs=2, space="PSUM") as ps_zp:
                for c in range(NCH):
                    t0 = c * TCH
                    # --- A: quantize (exact fp32, in place) + PE-transpose ---
                    xqT = gpool.tile([128, NH, TCH], f32r, tag="xqT")
                    for ttl in range(TCH // 128):
                        tt = t0 // 128 + ttl
                        tok = bigio.tile([128, H], f32, tag="tok")
                        nc.sync.dma_start(
                            out=tok, in_=tokens[tt * 128:(tt + 1) * 128, :])
                        amax = smalls.tile([128, NH], f32, tag="amax")
                        nc.vector.tensor_reduce(
                            amax, tok.rearrange("p (a b) -> p a b", b=128),
                            axis=AX.X, op=ALU.max, apply_absolute_value=True)
                        scl = smalls.tile([128, NH], f32, tag="scl")
                        nc.vector.tensor_scalar_mul(scl, amax, float(1.0 / QMAX))
                        rs = smalls.tile([128, NH], f32, tag="rs")
                        nc.vector.reciprocal(rs, scl)
                        for j in range(NH):
                            nc.scalar.activation(tok[:, j * 128:(j + 1) * 128],
                                                 tok[:, j * 128:(j + 1) * 128],
                                                 AF.Copy, bias=MAGIC,
                                                 scale=rs[:, j:j + 1])
                        nc.vector.tensor_scalar_add(tok, tok, -MAGIC)
                        for j in range(NH):
                            nc.scalar.activation(tok[:, j * 128:(j + 1) * 128],
                                                 tok[:, j * 128:(j + 1) * 128],
                                                 AF.Copy, scale=scl[:, j:j + 1])
                        for j in range(NH):
                            ptr = ps_tr.tile([128, 128], f32, tag="ptr")
                            nc.tensor.transpose(
                                ptr, tok[:, j * 128:(j + 1) * 128], ident)
                            nc.vector.tensor_copy(
                                xqT[:, j, ttl * 128:(ttl + 1) * 128], ptr)

                    # --- B: gate/up GEMMs (f32r) + SwiGLU -> gatedT ---
                    gatedT = gpool.tile([128, NI, TCH], f32r, tag="gatedT")
                    for i in range(NI):
                        wgi = wpool.tile([128, NH, 128], f32, tag="wgi")
                        nc.sync.dma_start(
                            out=wgi,
                            in_=wg[:, i * 128:(i + 1) * 128].rearrange(
                                "(a p) c -> p a c", p=128))
                        wgr = wpool.tile([128, NH, 128], f32r, tag="wgr")
                        nc.vector.tensor_copy(wgr, wgi)
                        pg = []
                        for sub in range(NSUB):
                            s0 = sub * 512
                            ps_g = ps_mm.tile([128, 512], f32, tag="ps_g")
                            for h in range(NH):
                                nc.tensor.matmul(ps_g, wgr[:, h, :],
                                                 xqT[:, h, s0:s0 + 512],
                                                 start=(h == 0),
                                                 stop=(h == NH - 1))
                            pg.append(ps_g)
                        wui = wpool.tile([128, NH, 128], f32, tag="wgi")
                        nc.sync.dma_start(
                            out=wui,
                            in_=wu[:, i * 128:(i + 1) * 128].rearrange(
                                "(a p) c -> p a c", p=128))
                        wur = wpool.tile([128, NH, 128], f32r, tag="wgr")
                        nc.scalar.copy(wur, wui)
                        for sub in range(NSUB):
                            s0 = sub * 512
                            ps_u = ps_mm.tile([128, 512], f32, tag="ps_u")
                            for h in range(NH):
                                nc.tensor.matmul(ps_u, wur[:, h, :],
                                                 xqT[:, h, s0:s0 + 512],
                                                 start=(h == 0),
                                                 stop=(h == NH - 1))
                            sil = post.tile([128, 512], f32, tag="sil")
                            nc.scalar.activation(sil, pg[sub], AF.Silu)
                            nc.vector.tensor_mul(gatedT[:, i, s0:s0 + 512],
                                                 sil, ps_u)

                    # --- C: down GEMM + combine quant + routing weight ---
                    for hc in range(H // 512):
                        wdq = []
                        for i in range(NI):
                            wdi = wpool.tile([128, 512], f32, tag="wdi",
                                             bufs=2)
                            nc.sync.dma_start(
                                out=wdi,
                                in_=wd[i * 128:(i + 1) * 128,
                                       hc * 512:(hc + 1) * 512])
                            wdr = wpool.tile([128, 512], f32r, tag="wdr",
                                             bufs=NI + 1)
                            if i % 2 == 0:
                                nc.vector.tensor_copy(wdr, wdi)
                            else:
                                nc.scalar.copy(wdr, wdi)
                            wdq.append(wdr)
                        for ttl in range(TCH // 128):
                            tt = t0 // 128 + ttl
                            psz = ps_zp.tile([128, 512], f32, tag="pszt")
                            for i in range(NI):
                                nc.tensor.matmul(
                                    psz,
                                    gatedT[:, i, ttl * 128:(ttl + 1) * 128],
                                    wdq[i],
                                    start=(i == 0), stop=(i == NI - 1))
                            zamax = smalls.tile([128, 4], f32, tag="zamax")
                            nc.vector.tensor_reduce(
                                zamax, psz.rearrange("p (a b) -> p a b", b=128),
                                axis=AX.X, op=ALU.max, apply_absolute_value=True)
                            zscl = smalls.tile([128, 4], f32, tag="zscl")
                            nc.vector.tensor_scalar_mul(zscl, zamax,
                                                        float(1.0 / QMAX))
                            zrs = smalls.tile([128, 4], f32, tag="zrs")
                            nc.vector.reciprocal(zrs, zscl)
                            zsw = smalls.tile([128, 4], f32, tag="zsw")
                            nc.vector.tensor_scalar(zsw, zscl,
                                                    w_all[:, tt:tt + 1],
                                                    None, op0=ALU.mult)
                            rz = post.tile([128, 512], f32, tag="rz")
                            for j in range(4):
                                nc.scalar.activation(
                                    rz[:, j * 128:(j + 1) * 128],
                                    psz[:, j * 128:(j + 1) * 128],
                                    AF.Copy, bias=MAGIC, scale=zrs[:, j:j + 1])
                            nc.vector.tensor_scalar_add(rz, rz, -MAGIC)
                            for j in range(4):
                                nc.scalar.activation(
                                    rz[:, j * 128:(j + 1) * 128],
                                    rz[:, j * 128:(j + 1) * 128],
                                    AF.Copy, scale=zsw[:, j:j + 1])
                            nc.sync.dma_start(
                                out=outp[tt * 128:(tt + 1) * 128,
                                         hc * 512:(hc + 1) * 512],
                                in_=rz)

    nc.compile()
    return nc


_NC_CACHE = {}
_LAST = {"exec_ns": None}
TRACE = False


def last_exec_ns():
    return _LAST["exec_ns"]


def kernel(tokens, w_router, w_gate, w_up, w_down):
    tokens = np.ascontiguousarray(tokens, dtype=np.float32)
    w_router = np.ascontiguousarray(w_router, dtype=np.float32)
    tokensT = np.ascontiguousarray(tokens.T)

    if "nc" not in _NC_CACHE:
        _NC_CACHE["nc"] = build()
    nc = _NC_CACHE["nc"]

    in_maps = []
    for e in range(E):
        onehot = np.zeros((1, E), dtype=np.float32)
        onehot[0, e] = 1.0
        in_maps.append(dict(
            tokens=tokens, tokensT=tokensT, wr=w_router,
            wg=np.ascontiguousarray(w_gate[e], dtype=np.float32),
            wu=np.ascontiguousarray(w_up[e], dtype=np.float32),
            wd=np.ascontiguousarray(w_down[e], dtype=np.float32),
            esel1=onehot))

    res = run_bass_kernel_spmd(nc, in_maps, core_ids=list(range(E)),
                               trace=TRACE)
    if res.exec_time_ns is not None:
        _LAST["exec_ns"] = res.exec_time_ns
    out = np.zeros((T, H), dtype=np.float64)
    for e in range(E):
        out += res.results[e]["outp"].astype(np.float64)
    aux = np.float32(res.results[0]["aux"][0, 0])
    return out.astype(np.float32), aux
